# revision 9
# baseline (speedup 1.0000x reference)
"""Trainium2 Bass kernel for nn_Decoder (30-step scan of a tiny transformer block).

Data-parallel over batch: 32768 rows -> 8 cores x 4096. Per core, feature-major
layout (features on SBUF partitions, batch on the free dim), batch tiled by 512
columns (one PSUM bank per matmul). The T=30 scan is fully unrolled; the only
cross-step dependency is the [3, B] state, kept in two ping-pong SBUF tiles.

Matmuls run as float32r (full-rate fp32 streaming at N>=256). LayerNorm mean /
variance are computed with a ones/384 stationary matrix, which lands the
statistics already broadcast across all 128 partitions (no [1, N] row ops).
rsqrt = exp(-0.5*ln(var+eps)) so the whole kernel uses one ACT table set
(natural_log_exp_and_others: ln, exp, relu, square, copy, identity).
elu(x) = relu(x) + min(exp(x)-1, 0).

Host-side (in kernel()): weights are transposed into lhsT layout, biases are
folded (bo' = bo + Wo@bv, b1' = b1 + W1@beta1, b2' = b2 + beta1,
bd1' = bd1 + Wd1@beta2, bs into init_hidden), and the per-step gate multiply is
folded into the plan tensor (rows [plan_t*gate; gate] against [Wp.T; bp]).
"""

import os
import numpy as np
from contextlib import ExitStack

B, T, D, FF, HID = 32768, 30, 384, 1024, 64
LN_EPS = 1e-5
NCORES = 8
BL = B // NCORES  # 4096 rows per core
TN = 512          # batch tile (one PSUM bank of fp32)
KD = D // 128     # 3 feature chunks
KF = FF // 128    # 8 FF chunks

_STATE = {}


def _build_nc(t_steps=T, bl=BL):
    import concourse.bass as bass
    import concourse.bacc as bacc
    import concourse.mybir as mybir
    import concourse.tile as tile

    f32 = mybir.dt.float32
    f32r = mybir.dt.float32r
    AF = mybir.ActivationFunctionType
    OP = mybir.AluOpType
    PSUM = bass.MemorySpace.PSUM

    nt = bl // TN

    nc = bacc.Bacc(trn_type="TRN2", target_bir_lowering=False, debug=False)

    # ---- DRAM tensors (names are the in_map keys) ----
    d_plan = nc.dram_tensor("planTg", [t_steps, 4, bl], f32r, kind="ExternalInput").ap()
    d_ih2 = nc.dram_tensor("ih2T", [D, bl], f32, kind="ExternalInput").ap()
    d_st0 = nc.dram_tensor("state0T", [3, bl], f32r, kind="ExternalInput").ap()
    d_wpg = nc.dram_tensor("wpg", [4, D], f32r, kind="ExternalInput").ap()
    d_wst = nc.dram_tensor("wst", [3, D], f32r, kind="ExternalInput").ap()
    d_wv = nc.dram_tensor("wv", [D, D], f32r, kind="ExternalInput").ap()
    d_wo = nc.dram_tensor("wo", [D, D], f32r, kind="ExternalInput").ap()
    d_w1 = nc.dram_tensor("w1", [D, FF], f32r, kind="ExternalInput").ap()
    d_w2 = nc.dram_tensor("w2", [FF, D], f32r, kind="ExternalInput").ap()
    d_wd1 = nc.dram_tensor("wd1", [D, HID], f32r, kind="ExternalInput").ap()
    d_wd2 = nc.dram_tensor("wd2", [HID, 3], f32r, kind="ExternalInput").ap()
    d_bo2 = nc.dram_tensor("bo2", [D, 1], f32, kind="ExternalInput").ap()
    d_b1f = nc.dram_tensor("b1f", [FF, 1], f32, kind="ExternalInput").ap()
    d_b21 = nc.dram_tensor("b21", [D, 1], f32, kind="ExternalInput").ap()
    d_g1 = nc.dram_tensor("g1v", [D, 1], f32, kind="ExternalInput").ap()
    d_g2 = nc.dram_tensor("g2v", [D, 1], f32, kind="ExternalInput").ap()
    d_bd1 = nc.dram_tensor("bd1f", [HID, 1], f32, kind="ExternalInput").ap()
    d_bd2 = nc.dram_tensor("bd2v", [3, 1], f32, kind="ExternalInput").ap()
    d_ones = nc.dram_tensor("onesW", [128, 128], f32r, kind="ExternalInput").ap()
    d_out = nc.dram_tensor("outT", [t_steps, 3, bl], f32r, kind="ExternalOutput").ap()

    with tile.TileContext(nc) as tc, ExitStack() as ctx:
        wp = ctx.enter_context(tc.tile_pool(name="w", bufs=1))

        def wtile(name, shape, src, dt_=f32):
            t_ = wp.tile(shape, dt_, tag=name, name=name)
            nc.sync.dma_start(t_[:], src)
            return t_

        wpg = wtile("wpg", [4, D], d_wpg[:, :], f32r)
        wst = wtile("wst", [3, D], d_wst[:, :], f32r)
        wv = [wtile(f"wv{k}", [128, D], d_wv[k * 128:(k + 1) * 128, :], f32r) for k in range(KD)]
        wo = [wtile(f"wo{k}", [128, D], d_wo[k * 128:(k + 1) * 128, :], f32r) for k in range(KD)]
        w1 = [wtile(f"w1_{k}", [128, FF], d_w1[k * 128:(k + 1) * 128, :], f32r) for k in range(KD)]
        w2 = [wtile(f"w2_{q}", [128, D], d_w2[q * 128:(q + 1) * 128, :], f32r) for q in range(KF)]
        wd1 = [wtile(f"wd1_{k}", [128, HID], d_wd1[k * 128:(k + 1) * 128, :], f32r) for k in range(KD)]
        wd2 = wtile("wd2", [HID, 3], d_wd2[:, :], f32r)
        bo2 = [wtile(f"bo2_{m}", [128, 1], d_bo2[m * 128:(m + 1) * 128, :]) for m in range(KD)]
        b1f = [wtile(f"b1f_{q}", [128, 1], d_b1f[q * 128:(q + 1) * 128, :]) for q in range(KF)]
        b21 = [wtile(f"b21_{m}", [128, 1], d_b21[m * 128:(m + 1) * 128, :]) for m in range(KD)]
        g1 = [wtile(f"g1_{m}", [128, 1], d_g1[m * 128:(m + 1) * 128, :]) for m in range(KD)]
        g2 = [wtile(f"g2_{m}", [128, 1], d_g2[m * 128:(m + 1) * 128, :]) for m in range(KD)]
        bd1f = wtile("bd1f", [HID, 1], d_bd1[:, :])
        bd2v = wtile("bd2v", [3, 1], d_bd2[:, :])

        ones = wtile("ones", [128, 128], d_ones[:, :], f32r)
        epsb = wp.tile([128, 1], f32, tag="epsb", name="epsb")
        nc.vector.memset(epsb[:], LN_EPS)
        zerob = wp.tile([128, 1], f32, tag="zerob", name="zerob")
        nc.vector.memset(zerob[:], 0.0)

        # persistent state buffer (updated in place each step)
        stA = wp.tile([3, bl], f32r, tag="stA", name="stA")
        nc.sync.dma_start(stA[:], d_st0[:, :])

        # working pools
        io = ctx.enter_context(tc.tile_pool(name="io", bufs=4))
        sp = ctx.enter_context(tc.tile_pool(name="sp", bufs=3))
        hp = ctx.enter_context(tc.tile_pool(name="hp", bufs=9))
        ep = ctx.enter_context(tc.tile_pool(name="ep", bufs=2))
        pp = ctx.enter_context(tc.tile_pool(name="pp", bufs=8, space="PSUM"))

        def ps_tile(parts=128):
            return pp.tile([parts, TN], f32, tag="ps", name="ps")

        for t in range(t_steps):
            cur = nxt = stA
            for n in range(nt):
                cs = slice(n * TN, (n + 1) * TN)

                pg = io.tile([4, TN], f32r, tag="pg", name="pg")
                nc.sync.dma_start(pg[:], d_plan[t, :, cs])
                ih = []
                for k in range(KD):
                    c = io.tile([128, TN], f32, tag="ih", name="ih")
                    nc.sync.dma_start(c[:], d_ih2[k * 128:(k + 1) * 128, cs])
                    ih.append(c)

                # x = Wpg.T@[plan*g; g] + Wst.T@state + (init_hidden + bs)
                xs = []
                for m in range(KD):
                    ms = slice(m * 128, (m + 1) * 128)
                    ps = ps_tile()
                    nc.tensor.matmul(ps[:], (wpg[:, ms]), (pg[:]), start=True, stop=False)
                    nc.tensor.matmul(ps[:], (wst[:, ms]), (cur[:, cs]), start=False, stop=True)
                    x = sp.tile([128, TN], f32r, tag="x", name="x")
                    nc.vector.tensor_tensor(x[:], ps[:], ih[m][:], OP.add)
                    xs.append(x)

                # v = Wv.T @ x   (bv folded into bo2)
                v0 = []
                for m in range(KD):
                    ms = slice(m * 128, (m + 1) * 128)
                    ps = ps_tile()
                    for k in range(KD):
                        nc.tensor.matmul(ps[:], (wv[k][:, ms]), (xs[k][:]),
                                         start=(k == 0), stop=(k == KD - 1))
                    v = sp.tile([128, TN], f32r, tag="v0", name="v0")
                    nc.scalar.copy(v[:], ps[:])
                    v0.append(v)

                # r = x + Wo.T @ v + bo2
                rs = []
                for m in range(KD):
                    ms = slice(m * 128, (m + 1) * 128)
                    ps = ps_tile()
                    for k in range(KD):
                        nc.tensor.matmul(ps[:], (wo[k][:, ms]), (v0[k][:]),
                                         start=(k == 0), stop=(k == KD - 1))
                    r = sp.tile([128, TN], f32r, tag="r", name="r")
                    nc.vector.scalar_tensor_tensor(r[:], ps[:], bo2[m][:], xs[m][:], OP.add, OP.add)
                    rs.append(r)

                def layernorm(rin, gw, tagp):
                    mps = ps_tile()
                    for k in range(KD):
                        nc.tensor.matmul(mps[:], (ones[:]), (rin[k][:]),
                                         start=(k == 0), stop=(k == KD - 1))
                    xc, sq = [], []
                    for m in range(KD):
                        c = sp.tile([128, TN], f32, tag=tagp + "xc", name=tagp + "xc")
                        nc.vector.tensor_tensor(c[:], rin[m][:], mps[:], OP.subtract)
                        xc.append(c)
                        s = sp.tile([128, TN], f32r, tag=tagp + "sq", name=tagp + "sq")
                        nc.scalar.activation(s[:], c[:], AF.Square, bias=zerob[:])
                        sq.append(s)
                    vps = ps_tile()
                    for k in range(KD):
                        nc.tensor.matmul(vps[:], (ones[:]), (sq[k][:]),
                                         start=(k == 0), stop=(k == KD - 1))
                    lnt = sp.tile([128, TN], f32, tag=tagp + "ln", name=tagp + "ln", bufs=2)
                    nc.scalar.activation(lnt[:], vps[:], AF.Ln, bias=epsb[:])
                    rstd = sp.tile([128, TN], f32, tag=tagp + "rs", name=tagp + "rs", bufs=2)
                    nc.scalar.activation(rstd[:], lnt[:], AF.Exp, bias=zerob[:], scale=-0.5)
                    ys = []
                    for m in range(KD):
                        y = sp.tile([128, TN], f32r, tag=tagp + "y", name=tagp + "y")
                        nc.vector.scalar_tensor_tensor(y[:], xc[m][:], gw[m][:], rstd[:],
                                                       OP.mult, OP.mult)
                        ys.append(y)
                    return ys

                y0 = layernorm(rs, g1, "a")

                # FFN: h1 = relu(W1.T@y0 + b1f); r2 = y0 + W2.T@h1 + b21
                h1 = []
                for q in range(KF):
                    qs = slice(q * 128, (q + 1) * 128)
                    ps = ps_tile()
                    for k in range(KD):
                        nc.tensor.matmul(ps[:], (w1[k][:, qs]), (y0[k][:]),
                                         start=(k == 0), stop=(k == KD - 1))
                    h = hp.tile([128, TN], f32r, tag="h1", name="h1")
                    nc.scalar.activation(h[:], ps[:], AF.Relu, bias=b1f[q][:])
                    h1.append(h)
                r2 = []
                for m in range(KD):
                    ms = slice(m * 128, (m + 1) * 128)
                    ps = ps_tile()
                    for q in range(KF):
                        nc.tensor.matmul(ps[:], (w2[q][:, ms]), (h1[q][:]),
                                         start=(q == 0), stop=(q == KF - 1))
                    rr = sp.tile([128, TN], f32r, tag="r2", name="r2")
                    nc.vector.scalar_tensor_tensor(rr[:], ps[:], b21[m][:], y0[m][:], OP.add, OP.add)
                    r2.append(rr)

                y2 = layernorm(r2, g2, "b")

                # decoder head: upd = Wd2.T @ elu(Wd1.T@y2 + bd1f) + bd2
                dps = ps_tile(HID)
                for k in range(KD):
                    nc.tensor.matmul(dps[:], (wd1[k][:]), (y2[k][:]),
                                     start=(k == 0), stop=(k == KD - 1))
                e1 = ep.tile([HID, TN], f32, tag="e1", name="e1")
                nc.scalar.activation(e1[:], dps[:], AF.Exp, bias=bd1f[:])
                rl = ep.tile([HID, TN], f32, tag="rl", name="rl")
                nc.scalar.activation(rl[:], dps[:], AF.Relu, bias=bd1f[:])
                eu = ep.tile([HID, TN], f32, tag="eu", name="eu")
                nc.vector.tensor_scalar(eu[:], e1[:], 1.0, 0.0, OP.subtract, OP.min)
                el = ep.tile([HID, TN], f32r, tag="el", name="el")
                nc.gpsimd.tensor_tensor(el[:], eu[:], rl[:], OP.add)

                d2 = ps_tile(3)
                nc.tensor.matmul(d2[:], (wd2[:]), (el[:]), start=True, stop=True)
                nc.vector.scalar_tensor_tensor(nxt[:, cs], d2[:], bd2v[:], cur[:, cs],
                                               OP.add, OP.add)
                nc.sync.dma_start(d_out[t, :, cs], nxt[:, cs])

    nc.compile()
    return nc


def _prep(inputs):
    """Host-side: fold biases, transpose weights to lhsT layout, shard batch."""
    g = {k: np.asarray(v, dtype=np.float32) for k, v in inputs.items()}
    Wv = g["Wqkv"][2 * D:, :]
    bv = g["bqkv"][2 * D:]

    col = lambda a: np.ascontiguousarray(a.reshape(-1, 1))
    shared = {
        "wpg": np.ascontiguousarray(np.concatenate([g["Wp"].T, g["bp"][None, :]], 0)),
        "wst": np.ascontiguousarray(g["Ws"].T),
        "wv": np.ascontiguousarray(Wv.T),
        "wo": np.ascontiguousarray(g["Wo"].T),
        "w1": np.ascontiguousarray(g["W1"].T),
        "w2": np.ascontiguousarray(g["W2"].T),
        "wd1": np.ascontiguousarray(g["Wd1"].T),
        "wd2": np.ascontiguousarray(g["Wd2"].T),
        "bo2": col(g["bo"] + g["Wo"] @ bv),
        "b1f": col(g["b1"] + g["W1"] @ g["beta1"]),
        "b21": col(g["b2"] + g["beta1"]),
        "g1v": col(g["g1"]),
        "g2v": col(g["g2"]),
        "bd1f": col(g["bd1"] + g["Wd1"] @ g["beta2"]),
        "bd2v": col(g["bd2"]),
        "onesW": np.full((128, 128), 1.0 / D, dtype=np.float32),
    }

    ih2 = (g["init_hidden"] + g["bs"][None, :]).T            # [D, B]
    gate = g["gate"][:, 0]                                    # [B]
    pgate = g["plan"] * g["gate"][:, None, :]                 # [B, T, 3]
    planT = pgate.transpose(1, 2, 0)                          # [T, 3, B]
    planTg = np.concatenate(
        [planT, np.broadcast_to(gate[None, None, :], (T, 1, B))], axis=1
    )                                                         # [T, 4, B]
    st0 = g["init_state"][:, :3].T                            # [3, B]

    in_maps = []
    for c in range(NCORES):
        cs = slice(c * BL, (c + 1) * BL)
        m = dict(shared)
        m["ih2T"] = np.ascontiguousarray(ih2[:, cs])
        m["planTg"] = np.ascontiguousarray(planTg[:, :, cs])
        m["state0T"] = np.ascontiguousarray(st0[:, cs])
        in_maps.append(m)
    return in_maps


def run(inputs, trace=False, trace_kwargs=None):
    from concourse.bass_utils import run_bass_kernel_spmd

    if "nc" not in _STATE:
        _STATE["nc"] = _build_nc()
    in_maps = _prep(inputs)
    res = run_bass_kernel_spmd(
        _STATE["nc"], in_maps, list(range(NCORES)), trace=trace,
        **(trace_kwargs or {}),
    )
    out = np.empty((B, T, 3), dtype=np.float32)
    for c in range(NCORES):
        outT = res.results[c]["outT"]                         # [T, 3, BL]
        out[c * BL:(c + 1) * BL] = outT.transpose(2, 0, 1)
    return out, res


def kernel(**inputs) -> np.ndarray:
    out, _ = run(inputs)
    return out


# revision 10
# speedup vs baseline: 1.0908x; 1.0908x over previous
"""Trainium2 Bass kernel for nn_Decoder (30-step scan of a tiny transformer block).

Data-parallel over batch: 32768 rows -> 8 cores x 4096. Per core, feature-major
layout (features on SBUF partitions, batch on the free dim), batch tiled by 512
columns (one PSUM bank per matmul). The T=30 scan is fully unrolled; the only
cross-step dependency is the [3, B] state, kept in two ping-pong SBUF tiles.

Matmuls run as float32r (full-rate fp32 streaming at N>=256). LayerNorm mean /
variance are computed with a ones/384 stationary matrix, which lands the
statistics already broadcast across all 128 partitions (no [1, N] row ops).
rsqrt = exp(-0.5*ln(var+eps)) so the whole kernel uses one ACT table set
(natural_log_exp_and_others: ln, exp, relu, square, copy, identity).
elu(x) = relu(x) + min(exp(x)-1, 0).

Host-side (in kernel()): weights are transposed into lhsT layout, biases are
folded (bo' = bo + Wo@bv, b1' = b1 + W1@beta1, b2' = b2 + beta1,
bd1' = bd1 + Wd1@beta2, bs into init_hidden), and the per-step gate multiply is
folded into the plan tensor (rows [plan_t*gate; gate] against [Wp.T; bp]).
"""

import os
import numpy as np
from contextlib import ExitStack

B, T, D, FF, HID = 32768, 30, 384, 1024, 64
LN_EPS = 1e-5
NCORES = 8
BL = B // NCORES  # 4096 rows per core
TN = 512          # batch tile (one PSUM bank of fp32)
KD = D // 128     # 3 feature chunks
KF = FF // 128    # 8 FF chunks

_STATE = {}


def _build_nc(t_steps=T, bl=BL):
    import concourse.bass as bass
    import concourse.bacc as bacc
    import concourse.mybir as mybir
    import concourse.tile as tile

    f32 = mybir.dt.float32
    f32r = mybir.dt.float32r
    bf16 = mybir.dt.bfloat16
    AF = mybir.ActivationFunctionType
    OP = mybir.AluOpType
    PSUM = bass.MemorySpace.PSUM

    nt = bl // TN

    nc = bacc.Bacc(trn_type="TRN2", target_bir_lowering=False, debug=False)

    # ---- DRAM tensors (names are the in_map keys) ----
    d_plan = nc.dram_tensor("planTg", [t_steps, 4, bl], f32r, kind="ExternalInput").ap()
    d_ih2 = nc.dram_tensor("ih2T", [D, bl], f32, kind="ExternalInput").ap()
    d_st0 = nc.dram_tensor("state0T", [3, bl], f32r, kind="ExternalInput").ap()
    d_wpg = nc.dram_tensor("wpg", [4, D], f32r, kind="ExternalInput").ap()
    d_wst = nc.dram_tensor("wst", [3, D], f32r, kind="ExternalInput").ap()
    d_wv = nc.dram_tensor("wv", [D, D], bf16, kind="ExternalInput").ap()
    d_wo = nc.dram_tensor("wo", [D, D], bf16, kind="ExternalInput").ap()
    d_w1 = nc.dram_tensor("w1", [D, FF], bf16, kind="ExternalInput").ap()
    d_w2 = nc.dram_tensor("w2", [FF, D], bf16, kind="ExternalInput").ap()
    d_wd1 = nc.dram_tensor("wd1", [D, HID], bf16, kind="ExternalInput").ap()
    d_wd2 = nc.dram_tensor("wd2", [HID, 3], bf16, kind="ExternalInput").ap()
    d_bo2 = nc.dram_tensor("bo2", [D, 1], f32, kind="ExternalInput").ap()
    d_b1f = nc.dram_tensor("b1f", [FF, 1], f32, kind="ExternalInput").ap()
    d_b21 = nc.dram_tensor("b21", [D, 1], f32, kind="ExternalInput").ap()
    d_g1 = nc.dram_tensor("g1v", [D, 1], f32, kind="ExternalInput").ap()
    d_g2 = nc.dram_tensor("g2v", [D, 1], f32, kind="ExternalInput").ap()
    d_bd1 = nc.dram_tensor("bd1f", [HID, 1], f32, kind="ExternalInput").ap()
    d_bd2 = nc.dram_tensor("bd2v", [3, 1], f32, kind="ExternalInput").ap()
    d_ones = nc.dram_tensor("onesW", [128, 128], f32r, kind="ExternalInput").ap()
    d_out = nc.dram_tensor("outT", [t_steps, 3, bl], f32r, kind="ExternalOutput").ap()

    with tile.TileContext(nc) as tc, ExitStack() as ctx:
        wp = ctx.enter_context(tc.tile_pool(name="w", bufs=1))

        def wtile(name, shape, src, dt_=f32):
            t_ = wp.tile(shape, dt_, tag=name, name=name)
            nc.sync.dma_start(t_[:], src)
            return t_

        wpg = wtile("wpg", [4, D], d_wpg[:, :], f32r)
        wst = wtile("wst", [3, D], d_wst[:, :], f32r)
        wv = [wtile(f"wv{k}", [128, D], d_wv[k * 128:(k + 1) * 128, :], bf16) for k in range(KD)]
        wo = [wtile(f"wo{k}", [128, D], d_wo[k * 128:(k + 1) * 128, :], bf16) for k in range(KD)]
        w1 = [wtile(f"w1_{k}", [128, FF], d_w1[k * 128:(k + 1) * 128, :], bf16) for k in range(KD)]
        w2 = [wtile(f"w2_{q}", [128, D], d_w2[q * 128:(q + 1) * 128, :], bf16) for q in range(KF)]
        wd1 = [wtile(f"wd1_{k}", [128, HID], d_wd1[k * 128:(k + 1) * 128, :], bf16) for k in range(KD)]
        wd2 = wtile("wd2", [HID, 3], d_wd2[:, :], bf16)
        bo2 = [wtile(f"bo2_{m}", [128, 1], d_bo2[m * 128:(m + 1) * 128, :]) for m in range(KD)]
        b1f = [wtile(f"b1f_{q}", [128, 1], d_b1f[q * 128:(q + 1) * 128, :]) for q in range(KF)]
        b21 = [wtile(f"b21_{m}", [128, 1], d_b21[m * 128:(m + 1) * 128, :]) for m in range(KD)]
        g1 = [wtile(f"g1_{m}", [128, 1], d_g1[m * 128:(m + 1) * 128, :]) for m in range(KD)]
        g2 = [wtile(f"g2_{m}", [128, 1], d_g2[m * 128:(m + 1) * 128, :]) for m in range(KD)]
        bd1f = wtile("bd1f", [HID, 1], d_bd1[:, :])
        bd2v = wtile("bd2v", [3, 1], d_bd2[:, :])

        ones = wtile("ones", [128, 128], d_ones[:, :], f32r)
        epsb = wp.tile([128, 1], f32, tag="epsb", name="epsb")
        nc.vector.memset(epsb[:], LN_EPS)
        zerob = wp.tile([128, 1], f32, tag="zerob", name="zerob")
        nc.vector.memset(zerob[:], 0.0)

        # persistent state buffer (updated in place each step)
        stA = wp.tile([3, bl], f32r, tag="stA", name="stA")
        nc.sync.dma_start(stA[:], d_st0[:, :])

        # working pools
        io = ctx.enter_context(tc.tile_pool(name="io", bufs=6))
        sp = ctx.enter_context(tc.tile_pool(name="sp", bufs=4))
        hp = ctx.enter_context(tc.tile_pool(name="hp", bufs=10))
        ep = ctx.enter_context(tc.tile_pool(name="ep", bufs=3))
        pp = ctx.enter_context(tc.tile_pool(name="pp", bufs=8, space="PSUM"))

        def ps_tile(parts=128):
            return pp.tile([parts, TN], f32, tag="ps", name="ps")

        for t in range(t_steps):
            cur = nxt = stA
            for n in range(nt):
                cs = slice(n * TN, (n + 1) * TN)

                pg = io.tile([4, TN], f32r, tag="pg", name="pg")
                nc.sync.dma_start(pg[:], d_plan[t, :, cs])
                ih = []
                for k in range(KD):
                    c = io.tile([128, TN], f32, tag="ih", name="ih")
                    nc.sync.dma_start(c[:], d_ih2[k * 128:(k + 1) * 128, cs])
                    ih.append(c)

                # x = Wpg.T@[plan*g; g] + Wst.T@state + (init_hidden + bs)
                xs = []
                for m in range(KD):
                    ms = slice(m * 128, (m + 1) * 128)
                    ps = ps_tile()
                    nc.tensor.matmul(ps[:], (wpg[:, ms]), (pg[:]), start=True, stop=False)
                    nc.tensor.matmul(ps[:], (wst[:, ms]), (cur[:, cs]), start=False, stop=True)
                    x = sp.tile([128, TN], bf16, tag="x", name="x")
                    nc.vector.tensor_tensor(x[:], ps[:], ih[m][:], OP.add)
                    xs.append(x)

                # v = Wv.T @ x   (bv folded into bo2)
                v0 = []
                for m in range(KD):
                    ms = slice(m * 128, (m + 1) * 128)
                    ps = ps_tile()
                    for k in range(KD):
                        nc.tensor.matmul(ps[:], (wv[k][:, ms]), (xs[k][:]),
                                         start=(k == 0), stop=(k == KD - 1))
                    v = sp.tile([128, TN], bf16, tag="v0", name="v0")
                    nc.scalar.copy(v[:], ps[:])
                    v0.append(v)

                # r = x + Wo.T @ v + bo2
                rs = []
                for m in range(KD):
                    ms = slice(m * 128, (m + 1) * 128)
                    ps = ps_tile()
                    for k in range(KD):
                        nc.tensor.matmul(ps[:], (wo[k][:, ms]), (v0[k][:]),
                                         start=(k == 0), stop=(k == KD - 1))
                    r = sp.tile([128, TN], f32r, tag="r", name="r")
                    nc.vector.scalar_tensor_tensor(r[:], ps[:], bo2[m][:], xs[m][:], OP.add, OP.add)
                    rs.append(r)

                def layernorm(rin, gw, tagp):
                    mps = ps_tile()
                    for k in range(KD):
                        nc.tensor.matmul(mps[:], (ones[:]), (rin[k][:]),
                                         start=(k == 0), stop=(k == KD - 1))
                    xc, sq = [], []
                    for m in range(KD):
                        c = sp.tile([128, TN], f32, tag=tagp + "xc", name=tagp + "xc")
                        nc.vector.tensor_tensor(c[:], rin[m][:], mps[:], OP.subtract)
                        xc.append(c)
                        s = sp.tile([128, TN], f32r, tag=tagp + "sq", name=tagp + "sq")
                        nc.gpsimd.tensor_tensor(s[:], c[:], c[:], OP.mult)
                        sq.append(s)
                    vps = ps_tile()
                    for k in range(KD):
                        nc.tensor.matmul(vps[:], (ones[:]), (sq[k][:]),
                                         start=(k == 0), stop=(k == KD - 1))
                    lnt = sp.tile([128, TN], f32, tag=tagp + "ln", name=tagp + "ln", bufs=2)
                    nc.scalar.activation(lnt[:], vps[:], AF.Ln, bias=epsb[:])
                    rstd = sp.tile([128, TN], f32, tag=tagp + "rs", name=tagp + "rs", bufs=2)
                    nc.scalar.activation(rstd[:], lnt[:], AF.Exp, bias=zerob[:], scale=-0.5)
                    ys = []
                    for m in range(KD):
                        y = sp.tile([128, TN], bf16, tag=tagp + "y", name=tagp + "y")
                        nc.vector.scalar_tensor_tensor(y[:], xc[m][:], gw[m][:], rstd[:],
                                                       OP.mult, OP.mult)
                        ys.append(y)
                    return ys

                y0 = layernorm(rs, g1, "a")

                # FFN: h1 = relu(W1.T@y0 + b1f); r2 = y0 + W2.T@h1 + b21
                h1 = []
                for q in range(KF):
                    qs = slice(q * 128, (q + 1) * 128)
                    ps = ps_tile()
                    for k in range(KD):
                        nc.tensor.matmul(ps[:], (w1[k][:, qs]), (y0[k][:]),
                                         start=(k == 0), stop=(k == KD - 1))
                    h = hp.tile([128, TN], bf16, tag="h1", name="h1")
                    nc.scalar.activation(h[:], ps[:], AF.Relu, bias=b1f[q][:])
                    h1.append(h)
                r2 = []
                for m in range(KD):
                    ms = slice(m * 128, (m + 1) * 128)
                    ps = ps_tile()
                    for q in range(KF):
                        nc.tensor.matmul(ps[:], (w2[q][:, ms]), (h1[q][:]),
                                         start=(q == 0), stop=(q == KF - 1))
                    rr = sp.tile([128, TN], f32r, tag="r2", name="r2")
                    nc.vector.scalar_tensor_tensor(rr[:], ps[:], b21[m][:], y0[m][:], OP.add, OP.add)
                    r2.append(rr)

                y2 = layernorm(r2, g2, "b")

                # decoder head: upd = Wd2.T @ elu(Wd1.T@y2 + bd1f) + bd2
                dps = ps_tile(HID)
                for k in range(KD):
                    nc.tensor.matmul(dps[:], (wd1[k][:]), (y2[k][:]),
                                     start=(k == 0), stop=(k == KD - 1))
                e1 = ep.tile([HID, TN], f32, tag="e1", name="e1")
                nc.scalar.activation(e1[:], dps[:], AF.Exp, bias=bd1f[:])
                rl = ep.tile([HID, TN], f32, tag="rl", name="rl")
                nc.scalar.activation(rl[:], dps[:], AF.Relu, bias=bd1f[:])
                eu = ep.tile([HID, TN], f32, tag="eu", name="eu")
                nc.vector.tensor_scalar(eu[:], e1[:], 1.0, 0.0, OP.subtract, OP.min)
                el = ep.tile([HID, TN], bf16, tag="el", name="el")
                nc.gpsimd.tensor_tensor(el[:], eu[:], rl[:], OP.add)

                d2 = ps_tile(3)
                nc.tensor.matmul(d2[:], (wd2[:]), (el[:]), start=True, stop=True)
                nc.vector.scalar_tensor_tensor(nxt[:, cs], d2[:], bd2v[:], cur[:, cs],
                                               OP.add, OP.add)
                nc.sync.dma_start(d_out[t, :, cs], nxt[:, cs])

    import concourse.bacc as bacc_mod
    if not getattr(bacc_mod, "_act_tables_patched", False):
        _orig_tables = bacc_mod.get_activation_tables
        _KEEP = "natural_log_exp_and_others"

        def _one_set_tables(arch):
            t = _orig_tables(arch)
            return {name: (fns if name == _KEEP else set()) for name, fns in t.items()}

        bacc_mod.get_activation_tables = _one_set_tables
        bacc_mod._act_tables_patched = True
    nc.compile()
    return nc


def _prep(inputs):
    """Host-side: fold biases, transpose weights to lhsT layout, shard batch."""
    g = {k: np.asarray(v, dtype=np.float32) for k, v in inputs.items()}
    Wv = g["Wqkv"][2 * D:, :]
    bv = g["bqkv"][2 * D:]

    import ml_dtypes
    b16 = lambda a: np.ascontiguousarray(a).astype(ml_dtypes.bfloat16)
    col = lambda a: np.ascontiguousarray(a.reshape(-1, 1))
    shared = {
        "wpg": np.ascontiguousarray(np.concatenate([g["Wp"].T, g["bp"][None, :]], 0)),
        "wst": np.ascontiguousarray(g["Ws"].T),
        "wv": b16(Wv.T),
        "wo": b16(g["Wo"].T),
        "w1": b16(g["W1"].T),
        "w2": b16(g["W2"].T),
        "wd1": b16(g["Wd1"].T),
        "wd2": b16(g["Wd2"].T),
        "bo2": col(g["bo"] + g["Wo"] @ bv),
        "b1f": col(g["b1"] + g["W1"] @ g["beta1"]),
        "b21": col(g["b2"] + g["beta1"]),
        "g1v": col(g["g1"]),
        "g2v": col(g["g2"]),
        "bd1f": col(g["bd1"] + g["Wd1"] @ g["beta2"]),
        "bd2v": col(g["bd2"]),
        "onesW": np.full((128, 128), 1.0 / D, dtype=np.float32),
    }

    ih2 = (g["init_hidden"] + g["bs"][None, :]).T            # [D, B]
    gate = g["gate"][:, 0]                                    # [B]
    pgate = g["plan"] * g["gate"][:, None, :]                 # [B, T, 3]
    planT = pgate.transpose(1, 2, 0)                          # [T, 3, B]
    planTg = np.concatenate(
        [planT, np.broadcast_to(gate[None, None, :], (T, 1, B))], axis=1
    )                                                         # [T, 4, B]
    st0 = g["init_state"][:, :3].T                            # [3, B]

    in_maps = []
    for c in range(NCORES):
        cs = slice(c * BL, (c + 1) * BL)
        m = dict(shared)
        m["ih2T"] = np.ascontiguousarray(ih2[:, cs])
        m["planTg"] = np.ascontiguousarray(planTg[:, :, cs])
        m["state0T"] = np.ascontiguousarray(st0[:, cs])
        in_maps.append(m)
    return in_maps


def run(inputs, trace=False, trace_kwargs=None):
    from concourse.bass_utils import run_bass_kernel_spmd

    if "nc" not in _STATE:
        _STATE["nc"] = _build_nc()
    in_maps = _prep(inputs)
    res = run_bass_kernel_spmd(
        _STATE["nc"], in_maps, list(range(NCORES)), trace=trace,
        **(trace_kwargs or {}),
    )
    out = np.empty((B, T, 3), dtype=np.float32)
    for c in range(NCORES):
        outT = res.results[c]["outT"]                         # [T, 3, BL]
        out[c * BL:(c + 1) * BL] = outT.transpose(2, 0, 1)
    return out, res


def kernel(**inputs) -> np.ndarray:
    out, _ = run(inputs)
    return out
